# revision 17
# baseline (speedup 1.0000x reference)
"""Dechirp-STFT Trainium2 kernel (mixed fp8/fp16 edition).

Computes, for each of D=16 chirp hypotheses, a resampled (linear-interp)
version of each Hann-windowed signal frame followed by a 1024-point FFT.

Strategy
--------
Per chirp d the whole frame-wise operation (gather/lerp resample -> DFT) is a
single linear map on the 1024-sample frame, so we fold both into one dense
matrix M_d built on the host from `dlnf`:  X_d = frames @ M_d.
Only rFFT bins f=0..512 are computed on device (input frames are real); device
rows use the packed-rfft column order [re0, re1, im1, ..., re511, im511,
re512] (1024 cols; im0/im512 are filled host-side).

Sharding: D axis across the 8 NeuronCores (2 chirps per core).

Precision: the PE runs fp8-e4m3 in DoubleRow mode (2 contraction-tiles per
pass, 2x rate) for the four k-tiles under the Hann window's edges
({0,1,6,7}: ~8% of the window energy, so fp8 quantization there costs only
~1e-2 relative error) and fp16 for the four center k-tiles {2,3,4,5}.
PE stream time: 2 chirps x 16 rowtiles x 2 coltiles x (2 DR + 4 fp16)
instructions = 0.75x the all-fp16 stream. Outputs are written as fp16
(values O(100), gate is 2e-2 — fp16 adds ~1e-4).

Dataflow: inputs are split across the Sync and Scalar HWDGE rings in the
exact order the PE consumes them (fp8 chirp-0 pieces first); the first four
row-tiles of chirp 0 are held as 8 one-bank PSUM accumulation groups whose
six stages chase the DMA arrivals stage-major, so the PE starts ~9us in and
never drains. Remaining row-tiles run group-major at full speed. A short
garbage-warmup burst on the Tensor engine un-throttles the PE clock before
the first real matmul. Output staged GRP=4 row-tiles in fp16 and written on
the Vector ring; the very last 4 row-tiles go per-(row,coltile) to shrink
the tail.
"""

import os
import numpy as np

K = 1024
HOP = 512
NW = 2047          # (1048576 - 1024) // 512 + 1
RWS = 2048         # padded row count (16 x 128 tiles)
NRT = RWS // 128   # 16 row tiles
GRP = 4            # row tiles per output staging group
NG = NRT // GRP
D = 16
NCORES = 8
DPC = D // NCORES  # chirps per core
NF = K // 2 + 1    # 513 rfft bins
OCOLS = 1024
CT = 512           # matmul col-tile (2 x 512 = 1024)
NCT = OCOLS // CT
KT = K // 128      # 8 contraction tiles
FP8_TILES = (0, 1, 6, 7)   # DoubleRow pairs (0,1) and (6,7)
FP16_TILES = (2, 3, 4, 5)
NWARM = 6

DEV_DT = os.environ.get("BASS_KERNEL_DT", "mixed")

_NC_CACHE = {}


def _build_pos(dlnf):
    """lo/frac per chirp, replicating the reference's fp32 op chain bit-exactly
    (jax-on-CPU); falls back to numpy fp32 if jax is unavailable."""
    try:
        import jax
        import jax.numpy as jnp

        with jax.default_device(jax.devices("cpu")[0]):
            betas = 2.0 * jnp.asarray(dlnf, dtype=jnp.float32)
            tau = jnp.linspace(0.0, 1.0, K)
            safe = jnp.abs(betas) < 1e-8
            betas_safe = jnp.where(safe, jnp.float32(1e-8), betas)
            eb = jnp.exp(betas_safe)
            t_source = 2.0 / betas_safe[:, None] * jnp.log1p(
                tau[None, :] * (eb[:, None] - 1.0)
            ) - 1.0
            identity = jnp.linspace(-1.0, 1.0, K)
            t_source = jnp.where(safe[:, None], identity[None, :], t_source)
            pos = np.asarray((t_source + 1.0) * 0.5 * (K - 1), dtype=np.float32)
            win = np.asarray(
                0.5 * (1.0 - jnp.cos(2.0 * jnp.pi * jnp.arange(K, dtype=jnp.float32) / K)),
                dtype=np.float32,
            )
    except Exception:
        d32 = np.asarray(dlnf, dtype=np.float32)
        betas = (np.float32(2.0) * d32).astype(np.float32)
        tau = np.linspace(0.0, 1.0, K, dtype=np.float32)
        safe = np.abs(betas) < np.float32(1e-8)
        betas_safe = np.where(safe, np.float32(1e-8), betas).astype(np.float32)
        eb = np.exp(betas_safe).astype(np.float32)
        t_source = (np.float32(2.0) / betas_safe)[:, None] * np.log1p(
            tau[None, :] * (eb[:, None] - np.float32(1.0))
        ).astype(np.float32) - np.float32(1.0)
        identity = np.linspace(-1.0, 1.0, K, dtype=np.float32)
        t_source = np.where(safe[:, None], identity[None, :], t_source).astype(np.float32)
        pos = ((t_source + np.float32(1.0)) * np.float32(0.5) * np.float32(K - 1)).astype(np.float32)
        n = np.arange(K, dtype=np.float32)
        win = (np.float32(0.5) * (np.float32(1.0) - np.cos(np.float32(2.0 * np.pi) * n / np.float32(K)))).astype(np.float32)

    lo = np.clip(pos.astype(np.int32), 0, K - 2)
    frac = (pos - lo.astype(np.float32)).astype(np.float32)
    return lo, frac, win


def _build_mats(dlnf):
    """(D, K, OCOLS) float32 combined interp+rDFT matrices, packed-rfft cols."""
    lo, frac, win = _build_pos(dlnf)
    n = np.arange(K, dtype=np.float64)
    f = np.arange(NF, dtype=np.float64)
    E = np.exp(-2j * np.pi * np.outer(n, f) / K)  # (K, NF) c128
    mats = np.empty((D, K, OCOLS), np.float32)
    for d in range(D):
        C = np.zeros((K, NF), np.complex128)
        np.add.at(C, lo[d], E * (1.0 - frac[d].astype(np.float64))[:, None])
        np.add.at(C, lo[d] + 1, E * frac[d].astype(np.float64)[:, None])
        mats[d, :, 0] = C.real[:, 0].astype(np.float32)
        mats[d, :, 1:-1:2] = C.real[:, 1:-1].astype(np.float32)
        mats[d, :, 2:-1:2] = C.imag[:, 1:-1].astype(np.float32)
        mats[d, :, -1] = C.real[:, -1].astype(np.float32)
    return mats, win


def _build_nc(dt_key):
    import concourse.mybir as mybir
    from concourse import bacc
    from concourse.tile import TileContext

    DR = mybir.MatmulPerfMode.DoubleRow
    f8 = mybir.dt.float8e4
    f16 = mybir.dt.float16
    f32 = mybir.dt.float32

    nc = bacc.Bacc(
        "TRN2", target_bir_lowering=False, debug=False, num_devices=NCORES
    )
    # partition-major layouts: dram[p, k, :] belongs to SBUF partition p.
    # wT8/m8 hold k-tiles (0,1,6,7) at indices (0,1,2,3); wT16/m16 hold
    # k-tiles (2,3,4,5) at indices (0,1,2,3).
    wT8 = nc.declare_dram_parameter("wT8", [128, 4, RWS], f8, isOutput=False)
    wT16 = nc.declare_dram_parameter("wT16", [128, 4, RWS], f16, isOutput=False)
    m8 = nc.declare_dram_parameter("m8", [DPC, 128, 4, OCOLS], f8, isOutput=False)
    m16 = nc.declare_dram_parameter("m16", [DPC, 128, 4, OCOLS], f16, isOutput=False)
    # output partition-major: out[c, p, t, :] = result row t*128+p of chirp c
    out = nc.declare_dram_parameter("out", [DPC, 128, NRT, OCOLS], f16, isOutput=True)

    with TileContext(nc) as tc:
        with (
            tc.tile_pool(name="wpool", bufs=1) as wpool,
            tc.tile_pool(name="mpool", bufs=1) as mpool,
            tc.tile_pool(name="opool", bufs=8) as opool,
            tc.tile_pool(name="pspool", bufs=2, space="PSUM") as pspool,
        ):
            # warm up the PE clock (HAM) while the input DMAs are in flight,
            # so the first real matmuls run at 2.4 GHz
            warm = wpool.tile([128, 512], f16, tag="warm", name="warm")
            nc.gpsimd.memset(warm[:], 0.0)
            wps = pspool.tile([128, 4, CT], f32, tag="ps", name="warmps")
            for _ in range(NWARM):
                nc.tensor.matmul(
                    wps[:, 0, :], warm[:, 0:128], warm[:, 0:512], start=True, stop=True
                )

            wt8 = wpool.tile([128, 4, RWS], f8, tag="w8", name="wt8")
            wt16 = wpool.tile([128, 4, RWS], f16, tag="w16", name="wt16")
            m8t = [
                mpool.tile([128, 4, OCOLS], f8, tag=f"m8_{c}", name=f"m8_{c}")
                for c in range(DPC)
            ]
            m16t = [
                mpool.tile([128, 4, OCOLS], f16, tag=f"m16_{c}", name=f"m16_{c}")
                for c in range(DPC)
            ]

            # ---- input DMA program, two HWDGE rings, in PE-consumption
            # order (gpsimd's ring is ~5x slower — never used). Early pieces
            # are small so the chase's first stages can start ~10.5us; sync
            # carries m8/m16-even + wt8 columns, scalar carries wt8 head +
            # wt16 (first 512 cols split per k-tile) + m16-odd.
            nc.sync.dma_start(out=m8t[0][:, 0:2, :], in_=m8[0][:, 0:2, :])
            nc.sync.dma_start(out=m8t[0][:, 2:4, :], in_=m8[0][:, 2:4, :])
            nc.sync.dma_start(out=m16t[0][:, 0, :], in_=m16[0][:, 0, :])   # k2
            nc.sync.dma_start(out=m16t[0][:, 2, :], in_=m16[0][:, 2, :])   # k4
            nc.sync.dma_start(out=wt8[:, :, 512:1024], in_=wT8[:, :, 512:1024])
            nc.sync.dma_start(out=wt8[:, :, 1024:2048], in_=wT8[:, :, 1024:2048])
            nc.sync.dma_start(out=m8t[1][:], in_=m8[1])
            nc.sync.dma_start(out=m16t[1][:, 1, :], in_=m16[1][:, 1, :])   # k3
            nc.sync.dma_start(out=m16t[1][:, 3, :], in_=m16[1][:, 3, :])   # k5

            nc.scalar.dma_start(out=wt8[:, 0:2, 0:512], in_=wT8[:, 0:2, 0:512])
            nc.scalar.dma_start(out=wt8[:, 2:4, 0:512], in_=wT8[:, 2:4, 0:512])
            nc.scalar.dma_start(out=wt16[:, 0, 0:512], in_=wT16[:, 0, 0:512])
            nc.scalar.dma_start(out=wt16[:, 1, 0:512], in_=wT16[:, 1, 0:512])
            nc.scalar.dma_start(out=m16t[0][:, 1, :], in_=m16[0][:, 1, :])  # k3
            nc.scalar.dma_start(out=wt16[:, 2, 0:512], in_=wT16[:, 2, 0:512])
            nc.scalar.dma_start(out=wt16[:, 3, 0:512], in_=wT16[:, 3, 0:512])
            nc.scalar.dma_start(out=m16t[0][:, 3, :], in_=m16[0][:, 3, :])  # k5
            nc.scalar.dma_start(out=wt16[:, :, 512:1024], in_=wT16[:, :, 512:1024])
            nc.scalar.dma_start(out=wt16[:, :, 1024:2048], in_=wT16[:, :, 1024:2048])
            nc.scalar.dma_start(out=m16t[1][:, 0, :], in_=m16[1][:, 0, :])  # k2
            nc.scalar.dma_start(out=m16t[1][:, 2, :], in_=m16[1][:, 2, :])  # k4

            # one accumulation stage of group (c, r, ct) into psum slice ps.
            # Stage ids: 0 = DR pair (0,1), 1 = DR pair (6,7), 2..5 = fp16
            # center k-tiles k2,k3,k4,k5 (wt16/m16 indices 0,1,2,3).
            def stage_mm(ps, c, r, ct, s, stop):
                rs = slice(r * 128, (r + 1) * 128)
                cs = slice(ct * CT, (ct + 1) * CT)
                if s == 0:
                    nc.tensor.matmul(
                        ps, wt8[:, 0:2, rs], m8t[c][:, 0:2, cs],
                        start=False, stop=stop, perf_mode=DR,
                        skip_group_check=True,
                    )
                elif s == 1:
                    nc.tensor.matmul(
                        ps, wt8[:, 2:4, rs], m8t[c][:, 2:4, cs],
                        start=False, stop=stop, perf_mode=DR,
                        skip_group_check=True,
                    )
                else:
                    kt = s - 2
                    nc.tensor.matmul(
                        ps, wt16[:, kt, rs], m16t[c][:, kt, cs],
                        start=False, stop=stop,
                        skip_group_check=True,
                    )

            # opening a group with a 1-column matmul on the zeroed warm tile
            # marks the whole 2KB bank pending-zero (ZERO_REGION granularity)
            # without paying the full-width zero pass on a real matmul
            def zstart(ps):
                nc.tensor.matmul(
                    ps[:, 0:1], warm[:, 0:128], warm[:, 0:1],
                    start=True, stop=False, skip_group_check=True,
                )

            # ---- chase: row-tiles 0..3 of chirp 0, stage-major across 8
            # accumulation groups (2 PSUM tiles x 4 bank-slices) so the PE
            # tracks the DMA arrivals, in arrival order: DR01, DR67, k2,
            # k3, k4, k5; stop on k5.
            CHASE_ORDER = (0, 1, 2, 3, 4, 5)
            pre_tiles = [
                pspool.tile([128, 4, CT], f32, tag="ps", name=f"psc_{h}")
                for h in range(2)
            ]
            def pre_slice(r, ct):
                return pre_tiles[r // 2][:, (r % 2) * 2 + ct, :]
            st0 = opool.tile([128, GRP, OCOLS], f16, tag="st", name="st0")
            for r in range(GRP):
                for ct in range(NCT):
                    zstart(pre_slice(r, ct))
            # skewed stage-major: tile0 (r0,r1) runs one stage ahead of
            # tile1 (r2,r3), so tile0's big copy hides under tile1's last
            # stage and the first main-loop group isn't blocked on it
            for si, s in enumerate(CHASE_ORDER):
                for ct in range(NCT):
                    stage_mm(pre_slice(0, ct), 0, 0, ct, s, stop=(si == 5))
                for ct in range(NCT):
                    stage_mm(pre_slice(1, ct), 0, 1, ct, s, stop=(si == 5))
                if si > 0:
                    sp = CHASE_ORDER[si - 1]
                    for ct in range(NCT):
                        stage_mm(pre_slice(2, ct), 0, 2, ct, sp, stop=False)
                    for ct in range(NCT):
                        stage_mm(pre_slice(3, ct), 0, 3, ct, sp, stop=False)
            nc.vector.tensor_copy(
                out=st0[:, 0:2, :].rearrange("p n x -> p (n x)"),
                in_=pre_tiles[0][:].rearrange("p n x -> p (n x)"),
            )
            sl = CHASE_ORDER[-1]
            for ct in range(NCT):
                stage_mm(pre_slice(2, ct), 0, 2, ct, sl, stop=True)
            for ct in range(NCT):
                stage_mm(pre_slice(3, ct), 0, 3, ct, sl, stop=True)
            nc.vector.tensor_copy(
                out=st0[:, 2:4, :].rearrange("p n x -> p (n x)"),
                in_=pre_tiles[1][:].rearrange("p n x -> p (n x)"),
            )
            nc.gpsimd.dma_start(out=out[0][:, 0:GRP, :], in_=st0[:, 0:GRP, :])

            # ---- remaining row-tiles, group-major ----
            for c in range(DPC):
                for g in range(NG):
                    if c == 0 and g == 0:
                        continue
                    last_grp = c == DPC - 1 and g == NG - 1
                    tiles = [
                        pspool.tile([128, 4, CT], f32, tag="ps", name=f"ps{c}_{g}_{h}")
                        for h in range(2)
                    ]
                    if not last_grp:
                        st = opool.tile([128, GRP, OCOLS], f16, tag="st", name=f"st{c}_{g}")
                        for rr in range(GRP):
                            r = g * GRP + rr
                            for ct in range(NCT):
                                ps = tiles[rr // 2][:, (rr % 2) * 2 + ct, :]
                                zstart(ps)
                                for s in range(6):
                                    stage_mm(ps, c, r, ct, s, stop=(s == 5))
                            if rr % 2 == 1:
                                h = rr // 2
                                nc.vector.tensor_copy(
                                    out=st[:, 2 * h:2 * h + 2, :].rearrange("p n x -> p (n x)"),
                                    in_=tiles[h][:].rearrange("p n x -> p (n x)"),
                                )
                        # the slow gpsimd ring drains big groups in the
                        # background; the very last big group goes on sync so
                        # it isn't stuck behind the gpsimd backlog at the end
                        ring = nc.sync if (c, g) == (DPC - 1, NG - 2) else nc.gpsimd
                        ring.dma_start(
                            out=out[c][:, g * GRP:(g + 1) * GRP, :], in_=st[:, 0:GRP, :]
                        )
                    else:
                        # final group: first 3 row-tiles staged together, the
                        # very last row-tile per-coltile on the (by now idle)
                        # sync/scalar rings so the post-matmul tail is short
                        st = opool.tile([128, GRP, OCOLS], f16, tag="st", name="stL")
                        for rr in range(GRP):
                            r = g * GRP + rr
                            for ct in range(NCT):
                                ps = tiles[rr // 2][:, (rr % 2) * 2 + ct, :]
                                zstart(ps)
                                for s in range(6):
                                    stage_mm(ps, c, r, ct, s, stop=(s == 5))
                                if rr == GRP - 1:
                                    # final row-tile: per-coltile copy + DMA on
                                    # the idle fast rings for a short tail
                                    nc.vector.tensor_copy(
                                        out=st[:, rr, ct * CT:(ct + 1) * CT], in_=ps
                                    )
                                    ring = nc.sync if ct == 0 else nc.scalar
                                    ring.dma_start(
                                        out=out[c][:, r:r + 1, ct * CT:(ct + 1) * CT],
                                        in_=st[:, rr:rr + 1, ct * CT:(ct + 1) * CT],
                                    )
                            if rr == 1:
                                nc.vector.tensor_copy(
                                    out=st[:, 0:2, :].rearrange("p n x -> p (n x)"),
                                    in_=tiles[0][:].rearrange("p n x -> p (n x)"),
                                )
                            elif rr == GRP - 2:
                                nc.vector.tensor_copy(
                                    out=st[:, 2, :], in_=tiles[1][:, 0:2, :].rearrange("p n x -> p (n x)")
                                )
                                nc.scalar.dma_start(
                                    out=out[c][:, g * GRP:g * GRP + 3, :],
                                    in_=st[:, 0:3, :],
                                )
    return nc


def _get_nc(dt_key):
    if dt_key not in _NC_CACHE:
        nc = _build_nc(dt_key)
        nc.finalize()
        _NC_CACHE[dt_key] = nc
    return _NC_CACHE[dt_key]


def _dev_arrays(x, dlnf, dt_key):
    """Returns (wT8, wT16, m8, m16) in device layouts."""
    import ml_dtypes

    f8 = ml_dtypes.float8_e4m3fn
    x = np.asarray(x)
    mats, win = _build_mats(np.asarray(dlnf))
    frames = np.lib.stride_tricks.sliding_window_view(x[0], K)[::HOP]  # (NW, K)
    frames = (frames * win).astype(np.float32)
    wT = np.zeros((K, RWS), np.float32)
    wT[:, :NW] = frames.T
    # partition-major: [128, KT, RWS] with [p, k, :] = wT[k*128+p, :]
    wT_pm = np.ascontiguousarray(wT.reshape(KT, 128, RWS).transpose(1, 0, 2))
    mats_pm = np.ascontiguousarray(
        mats.reshape(D, KT, 128, OCOLS).transpose(0, 2, 1, 3)
    )  # (D, 128, KT, OCOLS)
    wT8 = np.ascontiguousarray(wT_pm[:, FP8_TILES, :]).astype(f8)
    wT16 = np.ascontiguousarray(wT_pm[:, FP16_TILES, :]).astype(np.float16)
    m8 = np.ascontiguousarray(mats_pm[:, :, FP8_TILES, :]).astype(f8)
    m16 = np.ascontiguousarray(mats_pm[:, :, FP16_TILES, :]).astype(np.float16)
    return wT8, wT16, m8, m16


def _in_maps(x, dlnf, dt_key):
    wT8, wT16, m8, m16 = _dev_arrays(x, dlnf, dt_key)
    return [
        {
            "wT8": wT8,
            "wT16": wT16,
            "m8": np.ascontiguousarray(m8[i * DPC:(i + 1) * DPC]),
            "m16": np.ascontiguousarray(m16[i * DPC:(i + 1) * DPC]),
        }
        for i in range(NCORES)
    ]


def kernel(x, dlnf, n_hann_splits):
    assert int(n_hann_splits) == 1
    from concourse.bass_utils import run_bass_kernel_spmd

    dt_key = DEV_DT
    nc = _get_nc(dt_key)
    in_maps = _in_maps(x, dlnf, dt_key)

    core_ids = list(range(NCORES))
    res = run_bass_kernel_spmd(nc, in_maps, core_ids)

    out = np.empty((D, 1, NW, K), np.complex64)
    outv = out.view(np.float32).reshape(D, 1, NW, K, 2)  # (..., K, 2) re/im
    for i in core_ids:
        dev = res.results[i]["out"]  # (DPC, 128, NRT, OCOLS) f16, partition-major
        rows = np.ascontiguousarray(
            dev.transpose(0, 2, 1, 3)
        ).reshape(DPC, RWS, OCOLS)[:, :NW, :].astype(np.float32)
        sl = slice(i * DPC, (i + 1) * DPC)
        outv[sl, 0, :, 0, 0] = rows[:, :, 0]          # re0
        outv[sl, 0, :, 0, 1] = 0.0                    # im0
        outv[sl, 0, :, 1:NF - 1, :] = rows[:, :, 1:-1].reshape(DPC, NW, NF - 2, 2)
        outv[sl, 0, :, NF - 1, 0] = rows[:, :, -1]    # re512
        outv[sl, 0, :, NF - 1, 1] = 0.0               # im512
    out[:, :, :, NF:] = np.conj(out[:, :, :, 1:NF - 1][:, :, :, ::-1])
    return out


# revision 18
# speedup vs baseline: 1.0117x; 1.0117x over previous
"""Dechirp-STFT Trainium2 kernel (mixed fp8/fp16 edition).

Computes, for each of D=16 chirp hypotheses, a resampled (linear-interp)
version of each Hann-windowed signal frame followed by a 1024-point FFT.

Strategy
--------
Per chirp d the whole frame-wise operation (gather/lerp resample -> DFT) is a
single linear map on the 1024-sample frame, so we fold both into one dense
matrix M_d built on the host from `dlnf`:  X_d = frames @ M_d.
Only rFFT bins f=0..512 are computed on device (input frames are real); device
rows use the packed-rfft column order [re0, re1, im1, ..., re511, im511,
re512] (1024 cols; im0/im512 are filled host-side).

Sharding: D axis across the 8 NeuronCores (2 chirps per core).

Precision: the PE runs fp8-e4m3 in DoubleRow mode (2 contraction-tiles per
pass, 2x rate) for the four k-tiles under the Hann window's edges
({0,1,6,7}: ~8% of the window energy, so fp8 quantization there costs only
~1e-2 relative error) and fp16 for the four center k-tiles {2,3,4,5}.
PE stream time: 2 chirps x 16 rowtiles x 2 coltiles x (2 DR + 4 fp16)
instructions = 0.75x the all-fp16 stream. Outputs are written as fp16
(values O(100), gate is 2e-2 — fp16 adds ~1e-4).

Dataflow: inputs are split across the Sync and Scalar HWDGE rings in the
exact order the PE consumes them (fp8 chirp-0 pieces first); the first four
row-tiles of chirp 0 are held as 8 one-bank PSUM accumulation groups whose
six stages chase the DMA arrivals stage-major, so the PE starts ~9us in and
never drains. Remaining row-tiles run group-major at full speed. A short
garbage-warmup burst on the Tensor engine un-throttles the PE clock before
the first real matmul. Output staged GRP=4 row-tiles in fp16 and written on
the Vector ring; the very last 4 row-tiles go per-(row,coltile) to shrink
the tail.
"""

import os
import numpy as np

K = 1024
HOP = 512
NW = 2047          # (1048576 - 1024) // 512 + 1
RWS = 2048         # padded row count (16 x 128 tiles)
NRT = RWS // 128   # 16 row tiles
GRP = 4            # row tiles per output staging group
NG = NRT // GRP
D = 16
NCORES = 8
DPC = D // NCORES  # chirps per core
NF = K // 2 + 1    # 513 rfft bins
OCOLS = 1024
CT = 512           # matmul col-tile (2 x 512 = 1024)
NCT = OCOLS // CT
KT = K // 128      # 8 contraction tiles
FP8_TILES = (0, 1, 6, 7)   # DoubleRow pairs (0,1) and (6,7)
FP16_TILES = (2, 3, 4, 5)
NWARM = 6

DEV_DT = os.environ.get("BASS_KERNEL_DT", "mixed")

_NC_CACHE = {}


def _build_pos(dlnf):
    """lo/frac per chirp, replicating the reference's fp32 op chain bit-exactly
    (jax-on-CPU); falls back to numpy fp32 if jax is unavailable."""
    try:
        import jax
        import jax.numpy as jnp

        with jax.default_device(jax.devices("cpu")[0]):
            betas = 2.0 * jnp.asarray(dlnf, dtype=jnp.float32)
            tau = jnp.linspace(0.0, 1.0, K)
            safe = jnp.abs(betas) < 1e-8
            betas_safe = jnp.where(safe, jnp.float32(1e-8), betas)
            eb = jnp.exp(betas_safe)
            t_source = 2.0 / betas_safe[:, None] * jnp.log1p(
                tau[None, :] * (eb[:, None] - 1.0)
            ) - 1.0
            identity = jnp.linspace(-1.0, 1.0, K)
            t_source = jnp.where(safe[:, None], identity[None, :], t_source)
            pos = np.asarray((t_source + 1.0) * 0.5 * (K - 1), dtype=np.float32)
            win = np.asarray(
                0.5 * (1.0 - jnp.cos(2.0 * jnp.pi * jnp.arange(K, dtype=jnp.float32) / K)),
                dtype=np.float32,
            )
    except Exception:
        d32 = np.asarray(dlnf, dtype=np.float32)
        betas = (np.float32(2.0) * d32).astype(np.float32)
        tau = np.linspace(0.0, 1.0, K, dtype=np.float32)
        safe = np.abs(betas) < np.float32(1e-8)
        betas_safe = np.where(safe, np.float32(1e-8), betas).astype(np.float32)
        eb = np.exp(betas_safe).astype(np.float32)
        t_source = (np.float32(2.0) / betas_safe)[:, None] * np.log1p(
            tau[None, :] * (eb[:, None] - np.float32(1.0))
        ).astype(np.float32) - np.float32(1.0)
        identity = np.linspace(-1.0, 1.0, K, dtype=np.float32)
        t_source = np.where(safe[:, None], identity[None, :], t_source).astype(np.float32)
        pos = ((t_source + np.float32(1.0)) * np.float32(0.5) * np.float32(K - 1)).astype(np.float32)
        n = np.arange(K, dtype=np.float32)
        win = (np.float32(0.5) * (np.float32(1.0) - np.cos(np.float32(2.0 * np.pi) * n / np.float32(K)))).astype(np.float32)

    lo = np.clip(pos.astype(np.int32), 0, K - 2)
    frac = (pos - lo.astype(np.float32)).astype(np.float32)
    return lo, frac, win


def _build_mats(dlnf):
    """(D, K, OCOLS) float32 combined interp+rDFT matrices, packed-rfft cols."""
    lo, frac, win = _build_pos(dlnf)
    n = np.arange(K, dtype=np.float64)
    f = np.arange(NF, dtype=np.float64)
    E = np.exp(-2j * np.pi * np.outer(n, f) / K)  # (K, NF) c128
    mats = np.empty((D, K, OCOLS), np.float32)
    for d in range(D):
        C = np.zeros((K, NF), np.complex128)
        np.add.at(C, lo[d], E * (1.0 - frac[d].astype(np.float64))[:, None])
        np.add.at(C, lo[d] + 1, E * frac[d].astype(np.float64)[:, None])
        mats[d, :, 0] = C.real[:, 0].astype(np.float32)
        mats[d, :, 1:-1:2] = C.real[:, 1:-1].astype(np.float32)
        mats[d, :, 2:-1:2] = C.imag[:, 1:-1].astype(np.float32)
        mats[d, :, -1] = C.real[:, -1].astype(np.float32)
    return mats, win


def _build_nc(dt_key):
    import concourse.mybir as mybir
    from concourse import bacc
    from concourse.tile import TileContext

    DR = mybir.MatmulPerfMode.DoubleRow
    f8 = mybir.dt.float8e4
    f16 = mybir.dt.float16
    f32 = mybir.dt.float32

    nc = bacc.Bacc(
        "TRN2", target_bir_lowering=False, debug=False, num_devices=NCORES
    )
    # partition-major layouts: dram[p, k, :] belongs to SBUF partition p.
    # wT8/m8 hold k-tiles (0,1,6,7) at indices (0,1,2,3); wT16/m16 hold
    # k-tiles (2,3,4,5) at indices (0,1,2,3).
    wT8 = nc.declare_dram_parameter("wT8", [128, 4, RWS], f8, isOutput=False)
    wT16 = nc.declare_dram_parameter("wT16", [128, 4, RWS], f16, isOutput=False)
    m8 = nc.declare_dram_parameter("m8", [DPC, 128, 4, OCOLS], f8, isOutput=False)
    m16 = nc.declare_dram_parameter("m16", [DPC, 128, 4, OCOLS], f16, isOutput=False)
    # output partition-major: out[c, p, t, :] = result row t*128+p of chirp c
    out = nc.declare_dram_parameter("out", [DPC, 128, NRT, OCOLS], f16, isOutput=True)

    with TileContext(nc) as tc:
        with (
            tc.tile_pool(name="wpool", bufs=1) as wpool,
            tc.tile_pool(name="mpool", bufs=1) as mpool,
            tc.tile_pool(name="opool", bufs=8) as opool,
            tc.tile_pool(name="pspool", bufs=2, space="PSUM") as pspool,
        ):
            # warm up the PE clock (HAM) while the input DMAs are in flight,
            # so the first real matmuls run at 2.4 GHz
            warm = wpool.tile([128, 512], f16, tag="warm", name="warm")
            nc.gpsimd.memset(warm[:], 0.0)
            wps = pspool.tile([128, 4, CT], f32, tag="ps", name="warmps")
            for _ in range(NWARM):
                nc.tensor.matmul(
                    wps[:, 0, :], warm[:, 0:128], warm[:, 0:512], start=True, stop=True
                )

            wt8 = wpool.tile([128, 4, RWS], f8, tag="w8", name="wt8")
            wt16 = wpool.tile([128, 4, RWS], f16, tag="w16", name="wt16")
            m8t = [
                mpool.tile([128, 4, OCOLS], f8, tag=f"m8_{c}", name=f"m8_{c}")
                for c in range(DPC)
            ]
            m16t = [
                mpool.tile([128, 4, OCOLS], f16, tag=f"m16_{c}", name=f"m16_{c}")
                for c in range(DPC)
            ]

            # ---- input DMA program, two HWDGE rings, in PE-consumption
            # order (gpsimd's ring is ~5x slower — never used). Early pieces
            # are small so the chase's first stages can start ~10.5us; sync
            # carries m8/m16-even + wt8 columns, scalar carries wt8 head +
            # wt16 (first 512 cols split per k-tile) + m16-odd.
            nc.sync.dma_start(out=m8t[0][:, 0:2, :], in_=m8[0][:, 0:2, :])
            nc.sync.dma_start(out=m8t[0][:, 2:4, :], in_=m8[0][:, 2:4, :])
            nc.sync.dma_start(out=m16t[0][:, 0, :], in_=m16[0][:, 0, :])   # k2
            nc.sync.dma_start(out=m16t[0][:, 2, :], in_=m16[0][:, 2, :])   # k4
            nc.sync.dma_start(out=wt8[:, :, 512:1024], in_=wT8[:, :, 512:1024])
            nc.sync.dma_start(out=wt8[:, :, 1024:2048], in_=wT8[:, :, 1024:2048])
            nc.sync.dma_start(out=m8t[1][:], in_=m8[1])
            nc.sync.dma_start(out=m16t[1][:, 1, :], in_=m16[1][:, 1, :])   # k3
            nc.sync.dma_start(out=m16t[1][:, 3, :], in_=m16[1][:, 3, :])   # k5

            nc.scalar.dma_start(out=wt8[:, 0:2, 0:512], in_=wT8[:, 0:2, 0:512])
            nc.scalar.dma_start(out=wt8[:, 2:4, 0:512], in_=wT8[:, 2:4, 0:512])
            nc.scalar.dma_start(out=wt16[:, 0, 0:512], in_=wT16[:, 0, 0:512])
            nc.scalar.dma_start(out=wt16[:, 1, 0:512], in_=wT16[:, 1, 0:512])
            nc.scalar.dma_start(out=m16t[0][:, 1, :], in_=m16[0][:, 1, :])  # k3
            nc.scalar.dma_start(out=wt16[:, 2, 0:512], in_=wT16[:, 2, 0:512])
            nc.scalar.dma_start(out=wt16[:, 3, 0:512], in_=wT16[:, 3, 0:512])
            nc.scalar.dma_start(out=m16t[0][:, 3, :], in_=m16[0][:, 3, :])  # k5
            nc.scalar.dma_start(out=wt16[:, :, 512:1024], in_=wT16[:, :, 512:1024])
            nc.scalar.dma_start(out=wt16[:, :, 1024:2048], in_=wT16[:, :, 1024:2048])
            nc.scalar.dma_start(out=m16t[1][:, 0, :], in_=m16[1][:, 0, :])  # k2
            nc.scalar.dma_start(out=m16t[1][:, 2, :], in_=m16[1][:, 2, :])  # k4

            # one accumulation stage of group (c, r, ct) into psum slice ps.
            # Stage ids: 0 = DR pair (0,1), 1 = DR pair (6,7), 2..5 = fp16
            # center k-tiles k2,k3,k4,k5 (wt16/m16 indices 0,1,2,3).
            def stage_mm(ps, c, r, ct, s, stop, start=False):
                rs = slice(r * 128, (r + 1) * 128)
                cs = slice(ct * CT, (ct + 1) * CT)
                if s == 0:
                    nc.tensor.matmul(
                        ps, wt8[:, 0:2, rs], m8t[c][:, 0:2, cs],
                        start=start, stop=stop, perf_mode=DR,
                        skip_group_check=True,
                    )
                elif s == 1:
                    nc.tensor.matmul(
                        ps, wt8[:, 2:4, rs], m8t[c][:, 2:4, cs],
                        start=False, stop=stop, perf_mode=DR,
                        skip_group_check=True,
                    )
                else:
                    kt = s - 2
                    nc.tensor.matmul(
                        ps, wt16[:, kt, rs], m16t[c][:, kt, cs],
                        start=False, stop=stop,
                        skip_group_check=True,
                    )

            # opening a group with a 1-column matmul on the zeroed warm tile
            # marks the whole 2KB bank pending-zero (ZERO_REGION granularity)
            # without paying the full-width zero pass on a real matmul
            def zstart(ps):
                nc.tensor.matmul(
                    ps[:, 0:1], warm[:, 0:128], warm[:, 0:1],
                    start=True, stop=False, skip_group_check=True,
                )

            # ---- chase: row-tiles 0..3 of chirp 0, stage-major across 8
            # accumulation groups (2 PSUM tiles x 4 bank-slices) so the PE
            # tracks the DMA arrivals, in arrival order: DR01, DR67, k2,
            # k3, k4, k5; stop on k5.
            CHASE_ORDER = (0, 1, 2, 3, 4, 5)
            pre_tiles = [
                pspool.tile([128, 4, CT], f32, tag="ps", name=f"psc_{h}")
                for h in range(2)
            ]
            def pre_slice(r, ct):
                return pre_tiles[r // 2][:, (r % 2) * 2 + ct, :]
            st0 = opool.tile([128, GRP, OCOLS], f16, tag="st", name="st0")
            for r in range(GRP):
                for ct in range(NCT):
                    zstart(pre_slice(r, ct))
            # skewed stage-major: tile0 (r0,r1) runs one stage ahead of
            # tile1 (r2,r3), so tile0's big copy hides under tile1's last
            # stage and the first main-loop group isn't blocked on it
            for si, s in enumerate(CHASE_ORDER):
                for ct in range(NCT):
                    stage_mm(pre_slice(0, ct), 0, 0, ct, s, stop=(si == 5))
                for ct in range(NCT):
                    stage_mm(pre_slice(1, ct), 0, 1, ct, s, stop=(si == 5))
                if si > 0:
                    sp = CHASE_ORDER[si - 1]
                    for ct in range(NCT):
                        stage_mm(pre_slice(2, ct), 0, 2, ct, sp, stop=False)
                    for ct in range(NCT):
                        stage_mm(pre_slice(3, ct), 0, 3, ct, sp, stop=False)
            nc.vector.tensor_copy(
                out=st0[:, 0:2, :].rearrange("p n x -> p (n x)"),
                in_=pre_tiles[0][:].rearrange("p n x -> p (n x)"),
            )
            sl = CHASE_ORDER[-1]
            for ct in range(NCT):
                stage_mm(pre_slice(2, ct), 0, 2, ct, sl, stop=True)
            for ct in range(NCT):
                stage_mm(pre_slice(3, ct), 0, 3, ct, sl, stop=True)
            nc.vector.tensor_copy(
                out=st0[:, 2:4, :].rearrange("p n x -> p (n x)"),
                in_=pre_tiles[1][:].rearrange("p n x -> p (n x)"),
            )
            nc.gpsimd.dma_start(out=out[0][:, 0:GRP, :], in_=st0[:, 0:GRP, :])

            # ---- remaining row-tiles, group-major ----
            for c in range(DPC):
                for g in range(NG):
                    if c == 0 and g == 0:
                        continue
                    last_grp = c == DPC - 1 and g == NG - 1
                    tiles = [
                        pspool.tile([128, 4, CT], f32, tag="ps", name=f"ps{c}_{g}_{h}")
                        for h in range(2)
                    ]
                    if not last_grp:
                        st = opool.tile([128, GRP, OCOLS], f16, tag="st", name=f"st{c}_{g}")
                        for rr in range(GRP):
                            r = g * GRP + rr
                            for ct in range(NCT):
                                ps = tiles[rr // 2][:, (rr % 2) * 2 + ct, :]
                                for s in range(6):
                                    stage_mm(ps, c, r, ct, s, stop=(s == 5),
                                             start=(s == 0))
                            if rr % 2 == 1:
                                h = rr // 2
                                nc.vector.tensor_copy(
                                    out=st[:, 2 * h:2 * h + 2, :].rearrange("p n x -> p (n x)"),
                                    in_=tiles[h][:].rearrange("p n x -> p (n x)"),
                                )
                        # the slow gpsimd ring drains big groups in the
                        # background; the very last big group goes on sync so
                        # it isn't stuck behind the gpsimd backlog at the end
                        ring = nc.sync if (c, g) == (DPC - 1, NG - 2) else nc.gpsimd
                        ring.dma_start(
                            out=out[c][:, g * GRP:(g + 1) * GRP, :], in_=st[:, 0:GRP, :]
                        )
                    else:
                        # final group: first 3 row-tiles staged together, the
                        # very last row-tile per-coltile on the (by now idle)
                        # sync/scalar rings so the post-matmul tail is short
                        st = opool.tile([128, GRP, OCOLS], f16, tag="st", name="stL")
                        for rr in range(GRP):
                            r = g * GRP + rr
                            for ct in range(NCT):
                                ps = tiles[rr // 2][:, (rr % 2) * 2 + ct, :]
                                for s in range(6):
                                    stage_mm(ps, c, r, ct, s, stop=(s == 5),
                                             start=(s == 0))
                                if rr == GRP - 1:
                                    # final row-tile: per-coltile copy + DMA on
                                    # the idle fast rings for a short tail
                                    nc.vector.tensor_copy(
                                        out=st[:, rr, ct * CT:(ct + 1) * CT], in_=ps
                                    )
                                    ring = nc.sync if ct == 0 else nc.scalar
                                    ring.dma_start(
                                        out=out[c][:, r:r + 1, ct * CT:(ct + 1) * CT],
                                        in_=st[:, rr:rr + 1, ct * CT:(ct + 1) * CT],
                                    )
                            if rr == 1:
                                nc.vector.tensor_copy(
                                    out=st[:, 0:2, :].rearrange("p n x -> p (n x)"),
                                    in_=tiles[0][:].rearrange("p n x -> p (n x)"),
                                )
                            elif rr == GRP - 2:
                                nc.vector.tensor_copy(
                                    out=st[:, 2, :], in_=tiles[1][:, 0:2, :].rearrange("p n x -> p (n x)")
                                )
                                nc.scalar.dma_start(
                                    out=out[c][:, g * GRP:g * GRP + 3, :],
                                    in_=st[:, 0:3, :],
                                )
    return nc


def _get_nc(dt_key):
    if dt_key not in _NC_CACHE:
        nc = _build_nc(dt_key)
        nc.finalize()
        _NC_CACHE[dt_key] = nc
    return _NC_CACHE[dt_key]


def _dev_arrays(x, dlnf, dt_key):
    """Returns (wT8, wT16, m8, m16) in device layouts."""
    import ml_dtypes

    f8 = ml_dtypes.float8_e4m3fn
    x = np.asarray(x)
    mats, win = _build_mats(np.asarray(dlnf))
    frames = np.lib.stride_tricks.sliding_window_view(x[0], K)[::HOP]  # (NW, K)
    frames = (frames * win).astype(np.float32)
    wT = np.zeros((K, RWS), np.float32)
    wT[:, :NW] = frames.T
    # partition-major: [128, KT, RWS] with [p, k, :] = wT[k*128+p, :]
    wT_pm = np.ascontiguousarray(wT.reshape(KT, 128, RWS).transpose(1, 0, 2))
    mats_pm = np.ascontiguousarray(
        mats.reshape(D, KT, 128, OCOLS).transpose(0, 2, 1, 3)
    )  # (D, 128, KT, OCOLS)
    wT8 = np.ascontiguousarray(wT_pm[:, FP8_TILES, :]).astype(f8)
    wT16 = np.ascontiguousarray(wT_pm[:, FP16_TILES, :]).astype(np.float16)
    m8 = np.ascontiguousarray(mats_pm[:, :, FP8_TILES, :]).astype(f8)
    m16 = np.ascontiguousarray(mats_pm[:, :, FP16_TILES, :]).astype(np.float16)
    return wT8, wT16, m8, m16


def _in_maps(x, dlnf, dt_key):
    wT8, wT16, m8, m16 = _dev_arrays(x, dlnf, dt_key)
    return [
        {
            "wT8": wT8,
            "wT16": wT16,
            "m8": np.ascontiguousarray(m8[i * DPC:(i + 1) * DPC]),
            "m16": np.ascontiguousarray(m16[i * DPC:(i + 1) * DPC]),
        }
        for i in range(NCORES)
    ]


def kernel(x, dlnf, n_hann_splits):
    assert int(n_hann_splits) == 1
    from concourse.bass_utils import run_bass_kernel_spmd

    dt_key = DEV_DT
    nc = _get_nc(dt_key)
    in_maps = _in_maps(x, dlnf, dt_key)

    core_ids = list(range(NCORES))
    res = run_bass_kernel_spmd(nc, in_maps, core_ids)

    out = np.empty((D, 1, NW, K), np.complex64)
    outv = out.view(np.float32).reshape(D, 1, NW, K, 2)  # (..., K, 2) re/im
    for i in core_ids:
        dev = res.results[i]["out"]  # (DPC, 128, NRT, OCOLS) f16, partition-major
        rows = np.ascontiguousarray(
            dev.transpose(0, 2, 1, 3)
        ).reshape(DPC, RWS, OCOLS)[:, :NW, :].astype(np.float32)
        sl = slice(i * DPC, (i + 1) * DPC)
        outv[sl, 0, :, 0, 0] = rows[:, :, 0]          # re0
        outv[sl, 0, :, 0, 1] = 0.0                    # im0
        outv[sl, 0, :, 1:NF - 1, :] = rows[:, :, 1:-1].reshape(DPC, NW, NF - 2, 2)
        outv[sl, 0, :, NF - 1, 0] = rows[:, :, -1]    # re512
        outv[sl, 0, :, NF - 1, 1] = 0.0               # im512
    out[:, :, :, NF:] = np.conj(out[:, :, :, 1:NF - 1][:, :, :, ::-1])
    return out
